# revision 17
# baseline (speedup 1.0000x reference)
"""GQA causal attention (B=2, T=2048, C=2048, 32 Q heads, 8 KV heads) on 8
Trainium2 NeuronCores.

Sharding: tensor-parallel over KV-head groups. Core i owns KV head i and its
4 query heads: it computes q/k/v projections for its heads (256/64/64 output
channels), flash-style causal attention in scores-transposed layout, then the
cores AllGather the (normalized) attention output in head-major transposed
layout and each core computes a 256-row slice of the transposed final
projection. Host concatenates the row slices and transposes.

v2 changes vs the first working kernel:
  - all matmuls run in bf16 (f32 PSUM accumulation); xt/weights stream at
    half the HBM bytes
  - V natural layout comes from the DMA xbar transpose, not PE transposes
  - projection copies pack two heads per DVE op (kT/qT duplicated halves)
  - output projection is computed transposed (outT [CQ, BT]): N=512 moving
    dim, 256 matmuls instead of 512, contiguous row stores
  - the AllGather is split into 8 token chunks pipelined with attention
    (chunk k gathers while chunk k+1 computes; output projection for chunk
    k runs 3 chunks behind), removing the serial collective + projection
    tail
"""

import os
import sys

sys.path.insert(0, "/opt/trn_rl_repo")

import numpy as np
import ml_dtypes

# bisect switches (devloop only; default all-off = full v2 behavior)
# NOTE: the DMA xbar transpose (dma_start(..., transpose=True)) produces
# garbage on this hardware even though CoreSim models it correctly — V is
# transposed on the PE instead.
V2_SINGLE_AG = bool(int(os.environ.get("V2_SINGLE_AG", "0")))
V2_NO_PACK = bool(int(os.environ.get("V2_NO_PACK", "0")))

import concourse.bass as bass
import concourse.mybir as mybir
import concourse.tile as tile

P = 128
B, T, C = 2, 2048, 2048
BT = B * T            # 4096
NH, NKV = 32, 8
HD = C // NH          # 64
G = NH // NKV         # 4 q heads per kv head / per core
CQ = G * HD           # 256 q/out channels per core
KC = C // P           # 16 contraction chunks
TQ = 512              # t-chunk
NCORES = 8
NCH = B * (T // TQ)   # 8 token chunks (b, qc)
CLAG = 3              # output-projection chunk lag behind attention

f32 = mybir.dt.float32
f32r = mybir.dt.float32r
bf16 = mybir.dt.bfloat16
EXP = mybir.ActivationFunctionType.Exp
SCALE = float(HD) ** -0.5


def split_multi_waits(nc):
    """Walrus codegen allows only one sync-wait per engine instruction; move
    extras onto standalone same-engine EventSemaphore waits placed before."""
    for fn in nc.m.functions:
        for bb in fn.blocks:
            out = []
            for inst in bb.instructions:
                si = inst.sync_info
                if si is not None and si.on_wait and len(si.on_wait) > 1:
                    waits = list(si.on_wait)
                    for j, w in enumerate(waits[:-1]):
                        nop = mybir.InstEventSemaphore(
                            name=f"{inst.name}-ws{j}", ins=[], outs=[],
                            engine=inst.engine)
                        nop.sync_info = mybir.SyncInfo(on_wait=[w], on_update=[])
                        out.append(nop)
                    inst.sync_info = mybir.SyncInfo(
                        on_wait=[waits[-1]], on_update=list(si.on_update))
                out.append(inst)
            try:
                bb.instructions[:] = out
            except TypeError:
                bb.instructions.clear()
                bb.instructions.extend(out)


def build():
    nc = bass.Bass(num_devices=NCORES)

    xt_d = nc.dram_tensor("xt", [C, BT], bf16, kind="ExternalInput")
    wq_d = nc.dram_tensor("wq", [C, CQ], bf16, kind="ExternalInput")
    wkv_d = nc.dram_tensor("wkv", [C, P], bf16, kind="ExternalInput")
    wp_d = nc.dram_tensor("wp", [C, CQ], bf16, kind="ExternalInput")
    bpt_d = nc.dram_tensor("bpt", [P, 2], f32, kind="ExternalInput")
    mask_d = nc.dram_tensor("masks", [P, 4 * TQ], bf16, kind="ExternalInput")
    idn_d = nc.dram_tensor("ident", [P, P], bf16, kind="ExternalInput")
    ones_d = nc.dram_tensor("ones", [1, HD], f32r, kind="ExternalInput")
    out_d = nc.dram_tensor("out", [CQ, BT], f32, kind="ExternalOutput")

    with tile.TileContext(nc) as tc:
        with tc.tile_pool(name="res", bufs=1) as res, \
             tc.tile_pool(name="dram", bufs=1, space="DRAM") as dp:
            ones_sb = res.tile([1, HD], f32r)
            nc.sync.dma_start(ones_sb[:], ones_d[:, :])
            mask_sb = res.tile([P, 4 * TQ], bf16)
            nc.sync.dma_start(mask_sb[:], mask_d[:, :])
            wp_sb = res.tile([P, KC, CQ], bf16)
            nc.sync.dma_start(wp_sb[:], wp_d.rearrange("(o p) n -> p o n", p=P))
            bpt_sb = res.tile([P, 2], f32)
            nc.sync.dma_start(bpt_sb[:], bpt_d[:, :])

            idn_sb = res.tile([P, P], bf16)
            nc.sync.dma_start(idn_sb[:], idn_d[:, :])
            # long-lived activations (all bf16).  qT2[i] holds head 2i on
            # partitions 0-63 and head 2i+1 on partitions 64-127; kT2 holds
            # the same kT on both partition halves so either q half can
            # contract against it.
            if V2_NO_PACK:
                qTh = [res.tile([HD, BT], bf16, name=f"qt{h}") for h in range(G)]
                kT1 = res.tile([HD, BT], bf16)
            else:
                qT2 = [res.tile([P, BT], bf16, name=f"qt{i}") for i in range(2)]
                kT2 = res.tile([P, BT], bf16)
            va = res.tile([P, BT // P, HD + 2], bf16)  # v natural + ones col
            nc.vector.memset(va[:, :, HD:HD + 1], 1.0)
            nc.vector.memset(va[:, :, HD + 1:HD + 2], 0.0)
            yt_loc = dp.tile([NCH, CQ, TQ], bf16)
            yt_ag = [dp.tile([NCORES * CQ, TQ], bf16, addr_space="Shared",
                             name=f"ytag{c}") for c in range(NCH)]

            # ---- Phase 1: q/k/v projections (contract C on partitions) ----
            with tc.tile_pool(name="xp", bufs=3) as xp, \
                 tc.tile_pool(name="w1", bufs=1) as w1, \
                 tc.tile_pool(name="pps", bufs=1, space="PSUM") as pps:
                wq_sb = w1.tile([P, KC, CQ], bf16)
                nc.sync.dma_start(wq_sb[:], wq_d.rearrange("(o p) n -> p o n", p=P))
                wkv_sb = w1.tile([P, KC, P], bf16)
                nc.sync.dma_start(wkv_sb[:], wkv_d.rearrange("(o p) n -> p o n", p=P))
                for tb in range(BT // TQ):
                    xt_t = xp.tile([P, KC, TQ], bf16, tag="xt")
                    nc.sync.dma_start(
                        xt_t[:], xt_d.rearrange("(o p) t -> p o t", p=P)
                        [:, :, tb * TQ:(tb + 1) * TQ])
                    q0_ps = pps.tile([P, TQ], f32, tag="q0")
                    q1_ps = pps.tile([P, TQ], f32, tag="q1")
                    kv_ps = pps.tile([P, TQ], f32, tag="kv")
                    for c in range(KC):
                        nc.tensor.matmul(q0_ps[:], wq_sb[:, c, 0:P], xt_t[:, c, :],
                                         start=(c == 0), stop=(c == KC - 1))
                        nc.tensor.matmul(q1_ps[:], wq_sb[:, c, P:CQ], xt_t[:, c, :],
                                         start=(c == 0), stop=(c == KC - 1))
                        nc.tensor.matmul(kv_ps[:], wkv_sb[:, c, :], xt_t[:, c, :],
                                         start=(c == 0), stop=(c == KC - 1))
                    sl = slice(tb * TQ, (tb + 1) * TQ)
                    if V2_NO_PACK:
                        nc.vector.tensor_copy(qTh[0][:, sl], q0_ps[0:HD, :])
                        nc.vector.tensor_copy(qTh[1][:, sl], q0_ps[HD:P, :])
                        nc.vector.tensor_copy(qTh[2][:, sl], q1_ps[0:HD, :])
                        nc.vector.tensor_copy(qTh[3][:, sl], q1_ps[HD:P, :])
                        nc.vector.tensor_copy(kT1[:, sl], kv_ps[0:HD, :])
                    else:
                        nc.vector.tensor_copy(qT2[0][:, sl], q0_ps[:, :])
                        nc.vector.tensor_copy(qT2[1][:, sl], q1_ps[:, :])
                        nc.vector.tensor_copy(kT2[0:HD, sl], kv_ps[0:HD, :])
                        nc.vector.tensor_copy(kT2[HD:P, sl], kv_ps[0:HD, :])
                    vs_t = xp.tile([HD, TQ], bf16, tag="vs")
                    nc.vector.tensor_copy(vs_t[:], kv_ps[HD:P, :])
                    # V natural layout via PE transpose (the DMA xbar
                    # transpose is broken on this hardware)
                    for pg in range(TQ // P):
                        vt_ps = pps.tile([P, HD], bf16, tag="vt")
                        nc.tensor.transpose(
                            vt_ps[:], vs_t[:, pg * P:(pg + 1) * P],
                            idn_sb[0:HD, 0:HD])
                        nc.vector.tensor_copy(
                            va[:, tb * (TQ // P) + pg, 0:HD], vt_ps[:])

            # ---- Phase 2+3+4: causal attention, chunked AllGather, and the
            # transposed output projection, pipelined per (b, qc) chunk ----
            with tc.tile_pool(name="aps", bufs=2, space="PSUM") as aps, \
                 tc.tile_pool(name="yps", bufs=1, space="PSUM") as yps, \
                 tc.tile_pool(name="bps", bufs=1, space="PSUM") as bps, \
                 tc.tile_pool(name="ops", bufs=2, space="PSUM") as ops, \
                 tc.tile_pool(name="ep", bufs=3) as ep, \
                 tc.tile_pool(name="np_", bufs=2) as npo, \
                 tc.tile_pool(name="fp", bufs=2) as fp:

                def attn_chunk(cidx):
                    b, qc = cidx // 4, cidx % 4
                    nkb = 4 * qc + 4
                    yU = npo.tile([HD + 1, G, TQ], f32, tag="yU", bufs=2)
                    for h in range(G):
                        i2, h2 = h // 2, (h % 2) * HD
                        y_ps = yps.tile([HD + 2, TQ], f32, tag="y")
                        qsl = slice(b * T + qc * TQ, b * T + (qc + 1) * TQ)
                        if V2_NO_PACK:
                            qap = qTh[h][:, qsl]
                        else:
                            qap = qT2[i2][h2:h2 + HD, qsl]
                        for kbp in range(nkb // 2):
                            s_ps = aps.tile([P, 2, TQ], f32, tag="s")
                            for i in range(2):
                                kb = kbp * 2 + i
                                ksl = slice(b * T + kb * P, b * T + (kb + 1) * P)
                                kap = (kT1[:, ksl] if V2_NO_PACK
                                       else kT2[h2:h2 + HD, ksl])
                                nc.tensor.matmul(
                                    s_ps[:, i, :], kap,
                                    qap, start=True, stop=True)
                            ex = ep.tile([P, 2, TQ], bf16, tag="ex")
                            nc.scalar.activation(ex[:], s_ps[:], EXP, scale=SCALE)
                            for i in range(2):
                                kb = kbp * 2 + i
                                j = kb - 4 * qc
                                exh = ex[:, i, :]
                                if j >= 0:
                                    nc.vector.tensor_mul(
                                        exh, exh,
                                        mask_sb[:, j * TQ:(j + 1) * TQ])
                                nc.tensor.matmul(
                                    y_ps[:], va[:, b * (T // P) + kb, :], exh,
                                    start=(kb == 0), stop=(kb == nkb - 1))
                        nc.vector.tensor_copy(yU[:, h, :], y_ps[0:HD + 1, :])
                    # normalize: l rows -> partitions 0-3, reciprocal,
                    # broadcast back per head via ones-column matmul
                    lA = npo.tile([G, TQ], f32, tag="lA")
                    for h in range(G):
                        nc.sync.dma_start(lA[h:h + 1, :], yU[HD:HD + 1, h, :])
                    rA = npo.tile([G, TQ], f32, tag="rA")
                    nc.vector.reciprocal(rA[:], lA[:])
                    for h in range(G):
                        rrow = npo.tile([1, TQ], f32r, tag="rr")
                        nc.sync.dma_start(rrow[:], rA[h:h + 1, :].bitcast(f32r))
                        bc_ps = bps.tile([HD, TQ], f32, tag="bc")
                        nc.tensor.matmul(bc_ps[:], ones_sb[:], rrow[:],
                                         start=True, stop=True)
                        yn = npo.tile([HD, TQ], bf16, tag="yn")
                        nc.vector.tensor_mul(yn[:], yU[0:HD, h, :], bc_ps[:])
                        nc.sync.dma_start(
                            yt_loc[cidx][h * HD:(h + 1) * HD, :], yn[:])

                def ag_chunk(cidx):
                    nc.gpsimd.collective_compute(
                        "AllGather", mybir.AluOpType.bypass,
                        replica_groups=[list(range(NCORES))],
                        ins=[yt_loc[cidx].opt()], outs=[yt_ag[cidx][:].opt()])

                def proj_chunk(cidx):
                    yt_t = [fp.tile([P, 4, TQ], bf16, tag=f"yt{g}", name=f"yt{g}")
                            for g in range(4)]
                    src = yt_ag[cidx][:].rearrange("(g p) t -> p g t", p=P)
                    for g in range(4):
                        nc.sync.dma_start(yt_t[g][:], src[:, g * 4:(g + 1) * 4, :])
                    for half in range(2):
                        o_ps = ops.tile([P, TQ], f32, tag="o")
                        for c in range(KC):
                            nc.tensor.matmul(
                                o_ps[:], wp_sb[:, c, half * P:(half + 1) * P],
                                yt_t[c // 4][:, c % 4, :],
                                start=(c == 0), stop=(c == KC - 1))
                        o_sb = fp.tile([P, TQ], f32, tag="ob")
                        nc.vector.tensor_scalar_add(o_sb[:], o_ps[:],
                                                    bpt_sb[:, half:half + 1])
                        nc.sync.dma_start(
                            out_d[half * P:(half + 1) * P,
                                  cidx * TQ:(cidx + 1) * TQ], o_sb[:])

                # process longest chunks first within each batch so the tail
                # chunk is the cheapest one; lag the projection to keep the
                # gather off the critical path
                order = [b * 4 + qc for b in range(B)
                         for qc in range(3, -1, -1)]
                if V2_SINGLE_AG:
                    for cidx in order:
                        attn_chunk(cidx)
                    for cidx in order:
                        ag_chunk(cidx)
                    for cidx in order:
                        proj_chunk(cidx)
                else:
                    done = []
                    for n, cidx in enumerate(order):
                        attn_chunk(cidx)
                        ag_chunk(cidx)
                        done.append(cidx)
                        if n >= CLAG:
                            proj_chunk(done[n - CLAG])
                    for cidx in done[len(order) - CLAG:]:
                        proj_chunk(cidx)

    split_multi_waits(nc)
    return nc


_NC_CACHE = None


def _get_nc():
    global _NC_CACHE
    if _NC_CACHE is None:
        _NC_CACHE = build()
    return _NC_CACHE


def make_in_maps(x, wq, wk, wv, wp, bp):
    bfl = ml_dtypes.bfloat16
    x = np.asarray(x, dtype=np.float32)
    xt = np.ascontiguousarray(x.reshape(BT, C).T).astype(bfl)
    masks = np.zeros((P, 4 * TQ), dtype=np.float32)
    for j in range(4):
        kk = np.arange(P)[:, None]
        qq = np.arange(TQ)[None, :]
        masks[:, j * TQ:(j + 1) * TQ] = (j * P + kk <= qq).astype(np.float32)
    masks = masks.astype(bfl)
    ones = np.ones((1, HD), dtype=np.float32)
    in_maps = []
    for i in range(NCORES):
        cs = slice(i * CQ, (i + 1) * CQ)
        ks = slice(i * HD, (i + 1) * HD)
        wkv = np.concatenate(
            [np.asarray(wk)[:, ks], np.asarray(wv)[:, ks]], axis=1)
        bps = np.asarray(bp, np.float32)[cs]
        in_maps.append({
            "xt": xt,
            "wq": np.ascontiguousarray(np.asarray(wq, np.float32)[:, cs]).astype(bfl),
            "wkv": np.ascontiguousarray(wkv.astype(np.float32)).astype(bfl),
            "wp": np.ascontiguousarray(np.asarray(wp, np.float32)[:, cs]).astype(bfl),
            "bpt": np.ascontiguousarray(bps.reshape(2, P).T),
            "masks": masks,
            "ident": np.eye(P, dtype=np.float32).astype(bfl),
            "ones": ones,
        })
    return in_maps


def kernel(x, wq, wk, wv, wp, bp, _trace=False, _tmpdir=None):
    from concourse.bass_utils import run_bass_kernel_spmd
    nc = _get_nc()
    in_maps = make_in_maps(x, wq, wk, wv, wp, bp)
    res = run_bass_kernel_spmd(nc, in_maps, list(range(NCORES)), trace=_trace,
                               tmpdir=_tmpdir)
    out = np.concatenate([res.results[i]["out"] for i in range(NCORES)], axis=0)
    out = np.ascontiguousarray(out.T).reshape(B, T, C).astype(np.float32)
    if _trace:
        return out, res
    return out


# revision 22
# speedup vs baseline: 1.0194x; 1.0194x over previous
"""GQA causal attention (B=2, T=2048, C=2048, 32 Q heads, 8 KV heads) on 8
Trainium2 NeuronCores.

Sharding: tensor-parallel over KV-head groups. Core i owns KV head i and its
4 query heads: it computes q/k/v projections for its heads (256/64/64 output
channels), flash-style causal attention in scores-transposed layout, then the
cores AllGather the (normalized) attention output in head-major transposed
layout and each core computes a 256-row slice of the transposed final
projection. Host concatenates the row slices and transposes.

v2 changes vs the first working kernel:
  - all matmuls run in bf16 (f32 PSUM accumulation); xt/weights stream at
    half the HBM bytes
  - V natural layout comes from the DMA xbar transpose, not PE transposes
  - projection copies pack two heads per DVE op (kT/qT duplicated halves)
  - output projection is computed transposed (outT [CQ, BT]): N=512 moving
    dim, 256 matmuls instead of 512, contiguous row stores
  - the AllGather is split into 8 token chunks pipelined with attention
    (chunk k gathers while chunk k+1 computes; output projection for chunk
    k runs 3 chunks behind), removing the serial collective + projection
    tail
"""

import os
import sys

sys.path.insert(0, "/opt/trn_rl_repo")

import numpy as np
import ml_dtypes

# bisect switches (devloop only; default all-off = full v2 behavior)
# NOTE: the DMA xbar transpose (dma_start(..., transpose=True)) produces
# garbage on this hardware even though CoreSim models it correctly — V is
# transposed on the PE instead.
V2_SINGLE_AG = bool(int(os.environ.get("V2_SINGLE_AG", "0")))
V2_NO_PACK = bool(int(os.environ.get("V2_NO_PACK", "0")))

import concourse.bass as bass
import concourse.mybir as mybir
import concourse.tile as tile

P = 128
B, T, C = 2, 2048, 2048
BT = B * T            # 4096
NH, NKV = 32, 8
HD = C // NH          # 64
G = NH // NKV         # 4 q heads per kv head / per core
CQ = G * HD           # 256 q/out channels per core
KC = C // P           # 16 contraction chunks
TQ = 512              # t-chunk
NCORES = 8
NCH = B * (T // TQ)   # 8 token chunks (b, qc)
CLAG = 3              # output-projection chunk lag behind attention

f32 = mybir.dt.float32
f32r = mybir.dt.float32r
bf16 = mybir.dt.bfloat16
EXP = mybir.ActivationFunctionType.Exp
SCALE = float(HD) ** -0.5


def split_multi_waits(nc):
    """Walrus codegen allows only one sync-wait per engine instruction; move
    extras onto standalone same-engine EventSemaphore waits placed before."""
    for fn in nc.m.functions:
        for bb in fn.blocks:
            out = []
            for inst in bb.instructions:
                si = inst.sync_info
                if si is not None and si.on_wait and len(si.on_wait) > 1:
                    waits = list(si.on_wait)
                    for j, w in enumerate(waits[:-1]):
                        nop = mybir.InstEventSemaphore(
                            name=f"{inst.name}-ws{j}", ins=[], outs=[],
                            engine=inst.engine)
                        nop.sync_info = mybir.SyncInfo(on_wait=[w], on_update=[])
                        out.append(nop)
                    inst.sync_info = mybir.SyncInfo(
                        on_wait=[waits[-1]], on_update=list(si.on_update))
                out.append(inst)
            try:
                bb.instructions[:] = out
            except TypeError:
                bb.instructions.clear()
                bb.instructions.extend(out)


def build():
    nc = bass.Bass(num_devices=NCORES)

    xt_d = nc.dram_tensor("xt", [C, BT], bf16, kind="ExternalInput")
    wq_d = nc.dram_tensor("wq", [C, CQ], bf16, kind="ExternalInput")
    wkv_d = nc.dram_tensor("wkv", [C, P], bf16, kind="ExternalInput")
    wp_d = nc.dram_tensor("wp", [C, CQ], bf16, kind="ExternalInput")
    bpt_d = nc.dram_tensor("bpt", [P, 2], f32, kind="ExternalInput")
    mask_d = nc.dram_tensor("masks", [P, 4 * TQ], bf16, kind="ExternalInput")
    idn_d = nc.dram_tensor("ident", [P, P], bf16, kind="ExternalInput")
    ones_d = nc.dram_tensor("ones", [1, HD], f32r, kind="ExternalInput")
    out_d = nc.dram_tensor("out", [CQ, BT], f32, kind="ExternalOutput")

    with tile.TileContext(nc) as tc:
        with tc.tile_pool(name="res", bufs=1) as res, \
             tc.tile_pool(name="dram", bufs=1, space="DRAM") as dp:
            ones_sb = res.tile([1, HD], f32r)
            nc.sync.dma_start(ones_sb[:], ones_d[:, :])
            mask_sb = res.tile([P, 4 * TQ], bf16)
            nc.sync.dma_start(mask_sb[:], mask_d[:, :])
            wp_sb = res.tile([P, KC, CQ], bf16)
            nc.sync.dma_start(wp_sb[:], wp_d.rearrange("(o p) n -> p o n", p=P))
            bpt_sb = res.tile([P, 2], f32)
            nc.sync.dma_start(bpt_sb[:], bpt_d[:, :])

            idn_sb = res.tile([P, P], bf16)
            nc.sync.dma_start(idn_sb[:], idn_d[:, :])
            # long-lived activations (all bf16).  qT2[i] holds head 2i on
            # partitions 0-63 and head 2i+1 on partitions 64-127; kT2 holds
            # the same kT on both partition halves so either q half can
            # contract against it.
            if V2_NO_PACK:
                qTh = [res.tile([HD, BT], bf16, name=f"qt{h}") for h in range(G)]
                kT1 = res.tile([HD, BT], bf16)
            else:
                qT2 = [res.tile([P, BT], bf16, name=f"qt{i}") for i in range(2)]
                kT2 = res.tile([P, BT], bf16)
            va = res.tile([P, BT // P, HD + 2], bf16)  # v natural + ones col
            nc.vector.memset(va[:, :, HD:HD + 1], 1.0)
            nc.vector.memset(va[:, :, HD + 1:HD + 2], 0.0)
            yt_loc = dp.tile([NCH, CQ, TQ], bf16)
            yt_ag = [dp.tile([NCORES * CQ, TQ], bf16, addr_space="Shared",
                             name=f"ytag{c}") for c in range(NCH)]

            # ---- Phase 1: q/k/v projections (contract C on partitions) ----
            with tc.tile_pool(name="xp", bufs=3) as xp, \
                 tc.tile_pool(name="w1", bufs=1) as w1, \
                 tc.tile_pool(name="pps", bufs=1, space="PSUM") as pps:
                wq_sb = w1.tile([P, KC, CQ], bf16)
                nc.sync.dma_start(wq_sb[:], wq_d.rearrange("(o p) n -> p o n", p=P))
                wkv_sb = w1.tile([P, KC, P], bf16)
                nc.sync.dma_start(wkv_sb[:], wkv_d.rearrange("(o p) n -> p o n", p=P))
                for tb in range(BT // TQ):
                    xt_t = xp.tile([P, KC, TQ], bf16, tag="xt")
                    nc.sync.dma_start(
                        xt_t[:], xt_d.rearrange("(o p) t -> p o t", p=P)
                        [:, :, tb * TQ:(tb + 1) * TQ])
                    q0_ps = pps.tile([P, TQ], f32, tag="q0")
                    q1_ps = pps.tile([P, TQ], f32, tag="q1")
                    kv_ps = pps.tile([P, TQ], f32, tag="kv")
                    for c in range(KC):
                        nc.tensor.matmul(q0_ps[:], wq_sb[:, c, 0:P], xt_t[:, c, :],
                                         start=(c == 0), stop=(c == KC - 1))
                        nc.tensor.matmul(q1_ps[:], wq_sb[:, c, P:CQ], xt_t[:, c, :],
                                         start=(c == 0), stop=(c == KC - 1))
                        nc.tensor.matmul(kv_ps[:], wkv_sb[:, c, :], xt_t[:, c, :],
                                         start=(c == 0), stop=(c == KC - 1))
                    sl = slice(tb * TQ, (tb + 1) * TQ)
                    if V2_NO_PACK:
                        nc.vector.tensor_copy(qTh[0][:, sl], q0_ps[0:HD, :])
                        nc.vector.tensor_copy(qTh[1][:, sl], q0_ps[HD:P, :])
                        nc.vector.tensor_copy(qTh[2][:, sl], q1_ps[0:HD, :])
                        nc.vector.tensor_copy(qTh[3][:, sl], q1_ps[HD:P, :])
                        nc.vector.tensor_copy(kT1[:, sl], kv_ps[0:HD, :])
                    else:
                        nc.vector.tensor_copy(qT2[0][:, sl], q0_ps[:, :])
                        nc.vector.tensor_copy(qT2[1][:, sl], q1_ps[:, :])
                        nc.vector.tensor_copy(kT2[0:HD, sl], kv_ps[0:HD, :])
                        nc.vector.tensor_copy(kT2[HD:P, sl], kv_ps[0:HD, :])
                    vs_t = xp.tile([HD, TQ], bf16, tag="vs")
                    nc.vector.tensor_copy(vs_t[:], kv_ps[HD:P, :])
                    # V natural layout via PE transpose (the DMA xbar
                    # transpose is broken on this hardware)
                    for pg in range(TQ // P):
                        vt_ps = pps.tile([P, HD], bf16, tag="vt")
                        nc.tensor.transpose(
                            vt_ps[:], vs_t[:, pg * P:(pg + 1) * P],
                            idn_sb[0:HD, 0:HD])
                        nc.vector.tensor_copy(
                            va[:, tb * (TQ // P) + pg, 0:HD], vt_ps[:])

            # ---- Phase 2+3+4: causal attention, chunked AllGather, and the
            # transposed output projection, pipelined per (b, qc) chunk ----
            with tc.tile_pool(name="aps", bufs=2, space="PSUM") as aps, \
                 tc.tile_pool(name="yps", bufs=1, space="PSUM") as yps, \
                 tc.tile_pool(name="bps", bufs=1, space="PSUM") as bps, \
                 tc.tile_pool(name="ops", bufs=2, space="PSUM") as ops, \
                 tc.tile_pool(name="ep", bufs=3) as ep, \
                 tc.tile_pool(name="np_", bufs=2) as npo, \
                 tc.tile_pool(name="fp", bufs=2) as fp:

                def attn_chunk(cidx):
                    b, qc = cidx // 4, cidx % 4
                    nkb = 4 * qc + 4
                    yU = npo.tile([HD + 1, G, TQ], f32, tag="yU", bufs=2)
                    for h in range(G):
                        i2, h2 = h // 2, (h % 2) * HD
                        y_ps = yps.tile([HD + 2, TQ], f32, tag="y")
                        qsl = slice(b * T + qc * TQ, b * T + (qc + 1) * TQ)
                        if V2_NO_PACK:
                            qap = qTh[h][:, qsl]
                        else:
                            qap = qT2[i2][h2:h2 + HD, qsl]
                        # q columns below (2*kbp-4qc)*128 are fully masked
                        # for both strips of pair kbp: skip them in the
                        # scores matmuls, the exp, the mask multiplies, and
                        # the y matmuls.  Pairs are reordered (diagonal pairs
                        # in the middle) so the first and last y matmuls of
                        # the accumulation group are full width; for qc==0
                        # there is no trailing full pair, so no tightening.
                        if qc == 0:
                            pair_order = [0, 1]
                        else:
                            pair_order = ([0, 2 * qc, 2 * qc + 1]
                                          + list(range(1, 2 * qc)))
                        last_kb = pair_order[-1] * 2 + 1
                        for kbp in pair_order:
                            s_ps = aps.tile([P, 2, TQ], f32, tag="s")
                            e0 = (0 if qc == 0
                                  else max((kbp * 2 - 4 * qc) * P, 0))
                            q0s = [e0, e0]
                            for i in range(2):
                                kb = kbp * 2 + i
                                ksl = slice(b * T + kb * P, b * T + (kb + 1) * P)
                                kap = (kT1[:, ksl] if V2_NO_PACK
                                       else kT2[h2:h2 + HD, ksl])
                                nc.tensor.matmul(
                                    s_ps[:, i, q0s[i]:], kap,
                                    qap[:, q0s[i]:], start=True, stop=True)
                            ex = ep.tile([P, 2, TQ], bf16, tag="ex")
                            nc.scalar.activation(ex[:, :, e0:], s_ps[:, :, e0:],
                                                 EXP, scale=SCALE)
                            for i in range(2):
                                kb = kbp * 2 + i
                                j = kb - 4 * qc
                                exh = ex[:, i, :]
                                if j >= 0:
                                    nc.vector.tensor_mul(
                                        exh[:, q0s[i]:], exh[:, q0s[i]:],
                                        mask_sb[:, j * TQ + q0s[i]:(j + 1) * TQ])
                                nc.tensor.matmul(
                                    y_ps[:, q0s[i]:],
                                    va[:, b * (T // P) + kb, :], exh[:, q0s[i]:],
                                    start=(kb == 0), stop=(kb == last_kb),
                                    skip_group_check=(q0s[i] > 0))
                        nc.vector.tensor_copy(yU[:, h, :], y_ps[0:HD + 1, :])
                    # normalize: l rows -> partitions 0-3, reciprocal,
                    # broadcast back per head via ones-column matmul
                    lA = npo.tile([G, TQ], f32, tag="lA")
                    for h in range(G):
                        nc.sync.dma_start(lA[h:h + 1, :], yU[HD:HD + 1, h, :])
                    rA = npo.tile([G, TQ], f32, tag="rA")
                    nc.vector.reciprocal(rA[:], lA[:])
                    for h in range(G):
                        rrow = npo.tile([1, TQ], f32r, tag="rr")
                        nc.sync.dma_start(rrow[:], rA[h:h + 1, :].bitcast(f32r))
                        bc_ps = bps.tile([HD, TQ], f32, tag="bc")
                        nc.tensor.matmul(bc_ps[:], ones_sb[:], rrow[:],
                                         start=True, stop=True)
                        yn = npo.tile([HD, TQ], bf16, tag="yn")
                        nc.vector.tensor_mul(yn[:], yU[0:HD, h, :], bc_ps[:])
                        nc.sync.dma_start(
                            yt_loc[cidx][h * HD:(h + 1) * HD, :], yn[:])

                def ag_chunk(cidx):
                    nc.gpsimd.collective_compute(
                        "AllGather", mybir.AluOpType.bypass,
                        replica_groups=[list(range(NCORES))],
                        ins=[yt_loc[cidx].opt()], outs=[yt_ag[cidx][:].opt()])

                def proj_chunk(cidx):
                    yt_t = [fp.tile([P, 4, TQ], bf16, tag=f"yt{g}", name=f"yt{g}")
                            for g in range(4)]
                    src = yt_ag[cidx][:].rearrange("(g p) t -> p g t", p=P)
                    for g in range(4):
                        nc.sync.dma_start(yt_t[g][:], src[:, g * 4:(g + 1) * 4, :])
                    for half in range(2):
                        o_ps = ops.tile([P, TQ], f32, tag="o")
                        for c in range(KC):
                            nc.tensor.matmul(
                                o_ps[:], wp_sb[:, c, half * P:(half + 1) * P],
                                yt_t[c // 4][:, c % 4, :],
                                start=(c == 0), stop=(c == KC - 1))
                        o_sb = fp.tile([P, TQ], f32, tag="ob")
                        nc.vector.tensor_scalar_add(o_sb[:], o_ps[:],
                                                    bpt_sb[:, half:half + 1])
                        nc.sync.dma_start(
                            out_d[half * P:(half + 1) * P,
                                  cidx * TQ:(cidx + 1) * TQ], o_sb[:])

                # process longest chunks first within each batch so the tail
                # chunk is the cheapest one; lag the projection to keep the
                # gather off the critical path
                order = [b * 4 + qc for b in range(B)
                         for qc in range(3, -1, -1)]
                if V2_SINGLE_AG:
                    for cidx in order:
                        attn_chunk(cidx)
                    for cidx in order:
                        ag_chunk(cidx)
                    for cidx in order:
                        proj_chunk(cidx)
                else:
                    done = []
                    for n, cidx in enumerate(order):
                        attn_chunk(cidx)
                        ag_chunk(cidx)
                        done.append(cidx)
                        if n >= CLAG:
                            proj_chunk(done[n - CLAG])
                    for cidx in done[len(order) - CLAG:]:
                        proj_chunk(cidx)

    split_multi_waits(nc)
    return nc


_NC_CACHE = None


def _get_nc():
    global _NC_CACHE
    if _NC_CACHE is None:
        _NC_CACHE = build()
    return _NC_CACHE


def make_in_maps(x, wq, wk, wv, wp, bp):
    bfl = ml_dtypes.bfloat16
    x = np.asarray(x, dtype=np.float32)
    xt = np.ascontiguousarray(x.reshape(BT, C).T).astype(bfl)
    masks = np.zeros((P, 4 * TQ), dtype=np.float32)
    for j in range(4):
        kk = np.arange(P)[:, None]
        qq = np.arange(TQ)[None, :]
        masks[:, j * TQ:(j + 1) * TQ] = (j * P + kk <= qq).astype(np.float32)
    masks = masks.astype(bfl)
    ones = np.ones((1, HD), dtype=np.float32)
    in_maps = []
    for i in range(NCORES):
        cs = slice(i * CQ, (i + 1) * CQ)
        ks = slice(i * HD, (i + 1) * HD)
        wkv = np.concatenate(
            [np.asarray(wk)[:, ks], np.asarray(wv)[:, ks]], axis=1)
        bps = np.asarray(bp, np.float32)[cs]
        in_maps.append({
            "xt": xt,
            "wq": np.ascontiguousarray(np.asarray(wq, np.float32)[:, cs]).astype(bfl),
            "wkv": np.ascontiguousarray(wkv.astype(np.float32)).astype(bfl),
            "wp": np.ascontiguousarray(np.asarray(wp, np.float32)[:, cs]).astype(bfl),
            "bpt": np.ascontiguousarray(bps.reshape(2, P).T),
            "masks": masks,
            "ident": np.eye(P, dtype=np.float32).astype(bfl),
            "ones": ones,
        })
    return in_maps


def kernel(x, wq, wk, wv, wp, bp, _trace=False, _tmpdir=None):
    from concourse.bass_utils import run_bass_kernel_spmd
    nc = _get_nc()
    in_maps = make_in_maps(x, wq, wk, wv, wp, bp)
    res = run_bass_kernel_spmd(nc, in_maps, list(range(NCORES)), trace=_trace,
                               tmpdir=_tmpdir)
    out = np.concatenate([res.results[i]["out"] for i in range(NCORES)], axis=0)
    out = np.ascontiguousarray(out.T).reshape(B, T, C).astype(np.float32)
    if _trace:
        return out, res
    return out
